# revision 3
# baseline (speedup 1.0000x reference)
"""Trainium2 Bass kernel for nn_AlarmworkRNN: 2-track tanh RNN.

Math (per reference):
  in1 = X @ W_in1.T + b_in1 ; in2 = X @ W_in2.T + b_in2   (folded into recurrence)
  for l in 0..L-1:
      z1n = tanh(in1[l] + (z1 + z2) @ W_rec1.T)
      z2n = tanh(in2[l] + z2 @ W_rec2.T)  if l even else z2
      z1, z2 = z1n, z2n
  out = tanh(z1 @ W_out.T + b_out)       (computed on host, O=1)

Strategy: data-parallel over batch (8 cores x 64 rows). All matmuls in
float32r (~tf32 precision, full-rate moving stream). Recurrence state is held
transposed (z12T, z2T: [H=1024 -> 8 k-tiles of 128, B=64]) and used as the
matmul stationary; host-pretransposed weights are the moving operand, resident
in SBUF. The input projection X[l] @ W_in.T is folded into the same PSUM
accumulation as 2 extra k-tiles (stationary = host-pretransposed X[l].T), so
no vector-engine adds are needed before tanh. Per step: ACT tanh PSUM->SBUF,
8 PE transposes (z1n -> z1nT), one DVE add (z1nT + z2T -> z12T). The z2 path
(even steps) is computed one step early and used as PE fill for the
tanh/transpose/add latency of the serial z1 chain.
"""
import numpy as np

B, L, I, H = 512, 512, 256, 1024
NC = 8
BC = B // NC          # 64 batch rows per core
KH = H // 128         # 8 hidden k-tiles
KI = I // 128         # 2 input k-tiles

_CACHE = {}


def _build(L_steps, with_bias):
    import concourse.bacc as bacc
    import concourse.tile as tile
    import concourse.mybir as mybir

    F32, F32R = mybir.dt.float32, mybir.dt.float32r
    Tanh = mybir.ActivationFunctionType.Tanh
    Copy = mybir.ActivationFunctionType.Copy

    nc = bacc.Bacc("TRN2", target_bir_lowering=False)
    XT = nc.declare_dram_parameter("XT", [L_steps, I, BC], F32R, isOutput=False)
    W1T = nc.declare_dram_parameter("W1T", [H, H], F32R, isOutput=False)
    W2T = nc.declare_dram_parameter("W2T", [H, H], F32R, isOutput=False)
    Wi1T = nc.declare_dram_parameter("Wi1T", [I, H], F32R, isOutput=False)
    Wi2T = nc.declare_dram_parameter("Wi2T", [I, H], F32R, isOutput=False)
    IDN = nc.declare_dram_parameter("IDN", [64, 64], F32R, isOutput=False)
    ZRO = nc.declare_dram_parameter("ZRO", [128, KH * BC], F32R, isOutput=False)
    if with_bias:
        BIA = nc.declare_dram_parameter("BIA", [2, H], F32R, isOutput=False)
        ONE = nc.declare_dram_parameter("ONE", [1, BC], F32R, isOutput=False)
    OUT = nc.declare_dram_parameter("OUT", [BC, H], F32R, isOutput=True)

    with tile.TileContext(nc) as tc:
        with tc.tile_pool(name="const", bufs=1) as cpool, \
             tc.tile_pool(name="xt", bufs=6) as xpool, \
             tc.tile_pool(name="st", bufs=3) as spool, \
             tc.tile_pool(name="actt", bufs=3) as apool, \
             tc.tile_pool(name="ps1", bufs=2, space="PSUM") as ps1pool, \
             tc.tile_pool(name="ps2", bufs=1, space="PSUM") as ps2pool, \
             tc.tile_pool(name="pst", bufs=2, space="PSUM") as pstpool:

            # ---- resident weights: [128, ktile*H] with ktile-major free layout
            w1t_sb = cpool.tile([128, KH * H], F32R)
            w2t_sb = cpool.tile([128, KH * H], F32R)
            wi1t_sb = cpool.tile([128, KI * H], F32R)
            wi2t_sb = cpool.tile([128, KI * H], F32R)
            id_sb = cpool.tile([64, 64], F32R)
            nc.sync.dma_start(id_sb[:], IDN[:])
            for k in range(KH):
                nc.sync.dma_start(w1t_sb[:, k*H:(k+1)*H], W1T[k*128:(k+1)*128, :])
                nc.sync.dma_start(w2t_sb[:, k*H:(k+1)*H], W2T[k*128:(k+1)*128, :])
            for k in range(KI):
                nc.sync.dma_start(wi1t_sb[:, k*H:(k+1)*H], Wi1T[k*128:(k+1)*128, :])
                nc.sync.dma_start(wi2t_sb[:, k*H:(k+1)*H], Wi2T[k*128:(k+1)*128, :])
            if with_bias:
                bia_sb = cpool.tile([2, H], F32R)
                one_sb = cpool.tile([1, BC], F32R)
                nc.sync.dma_start(bia_sb[:], BIA[:])
                nc.sync.dma_start(one_sb[:], ONE[:])

            # ---- initial state (zeros, DMA'd so the tiles are f32r-typed producers)
            z12T = spool.tile([128, KH * BC], F32R, tag="z12T")
            z2T = spool.tile([128, KH * BC], F32R, tag="z2T")
            nc.sync.dma_start(z12T[:], ZRO[:])
            nc.sync.dma_start(z2T[:], ZRO[:])

            # ---- XT prefetch
            xts = {}

            def fetch_xt(l):
                if l >= L_steps:
                    return
                t = xpool.tile([128, KI * BC], F32R, tag="xt")
                for k in range(KI):
                    nc.sync.dma_start(t[:, k*BC:(k+1)*BC], XT[l, k*128:(k+1)*128, :])
                xts[l] = t

            def mm_x(ps, xt_t, wi_sb, bias_row):
                """Open the accumulation group: X[l] @ Wi.T (+ bias)."""
                for k in range(KI):
                    for b in range(2):
                        nc.tensor.matmul(
                            ps[0:BC, b*512:(b+1)*512],
                            xt_t[:, k*BC:(k+1)*BC],
                            wi_sb[:, k*H + b*512 : k*H + b*512 + 512],
                            start=(k == 0), stop=False)
                if with_bias:
                    for b in range(2):
                        nc.tensor.matmul(
                            ps[0:BC, b*512:(b+1)*512],
                            one_sb[0:1, :],
                            bia_sb[bias_row:bias_row+1, b*512:(b+1)*512],
                            start=False, stop=False)

            def mm_z(ps, zT, w_sb):
                """Close the accumulation group: + z @ W.T."""
                for k in range(KH):
                    for b in range(2):
                        nc.tensor.matmul(
                            ps[0:BC, b*512:(b+1)*512],
                            zT[:, k*BC:(k+1)*BC],
                            w_sb[:, k*H + b*512 : k*H + b*512 + 512],
                            start=False, stop=(k == KH - 1))

            def z2_path(l_target, z2T_in):
                """Full z2 update for even step l_target: matmul group + tanh +
                transposes + copy; returns the new z2T tile (pending)."""
                ps2 = ps2pool.tile([BC, H], F32, tag="ps2")
                mm_x(ps2, xts[l_target], wi2t_sb, 1)
                mm_z(ps2, z2T_in, w2t_sb)
                z2n = apool.tile([BC, H], F32R, tag="z2n")
                for c in range(2):
                    nc.scalar.activation(z2n[:, c*512:(c+1)*512], ps2[0:BC, c*512:(c+1)*512], Tanh)
                pst2 = pstpool.tile([128, KH * BC], F32R, tag="pst")
                for k in range(KH):
                    nc.tensor.transpose(pst2[:, k*BC:(k+1)*BC], z2n[:, k*128:(k+1)*128], id_sb[:])
                z2T_new = spool.tile([128, KH * BC], F32R, tag="z2T")
                nc.scalar.activation(z2T_new[:], pst2[:], Copy)
                return z2T_new

            # ---- prologue: prime XT, open step-0 z1 group, full step-0 z2 path
            for l in range(min(3, L_steps)):
                fetch_xt(l)
            ps1 = ps1pool.tile([BC, H], F32, tag="ps1")
            mm_x(ps1, xts[0], wi1t_sb, 0)
            z2T_pending = z2_path(0, z2T)

            z1n_final = None
            for l in range(L_steps):
                even = (l % 2 == 0)
                last = (l == L_steps - 1)
                fetch_xt(l + 3)

                # close this step's z1 accumulation (state entering step l)
                mm_z(ps1, z12T, w1t_sb)

                # z2 state after step l: updated on even steps
                if even:
                    z2T = z2T_pending

                # z2 path for step l+1 (if even), emitted early as PE fill;
                # uses z2 state after step l (== entering step l+1).
                if not last and (l + 1) % 2 == 0:
                    z2T_pending = z2_path(l + 1, z2T)

                # tanh of this step's z1
                z1n = apool.tile([BC, H], F32R, tag="z1n")
                for c in range(2):
                    nc.scalar.activation(z1n[:, c*512:(c+1)*512], ps1[0:BC, c*512:(c+1)*512], Tanh)
                if last:
                    z1n_final = z1n
                    break

                # open next step's z1 group (independent fill before the transposes)
                ps1 = ps1pool.tile([BC, H], F32, tag="ps1")
                mm_x(ps1, xts[l + 1], wi1t_sb, 0)

                # transpose z1n and form z12T = z1nT + z2T(after this step)
                pst1 = pstpool.tile([128, KH * BC], F32R, tag="pst")
                for k in range(KH):
                    nc.tensor.transpose(pst1[:, k*BC:(k+1)*BC], z1n[:, k*128:(k+1)*128], id_sb[:])
                z12T = spool.tile([128, KH * BC], F32R, tag="z12T")
                for c in range(2):
                    nc.vector.tensor_add(z12T[:, c*256:(c+1)*256], pst1[:, c*256:(c+1)*256], z2T[:, c*256:(c+1)*256])

                if l >= 1:
                    xts.pop(l - 1, None)

            nc.sync.dma_start(OUT[:], z1n_final[:])
    nc.compile()
    return nc


def _get_nc(L_steps, with_bias):
    key = (L_steps, with_bias)
    if key not in _CACHE:
        _CACHE[key] = _build(L_steps, with_bias)
    return _CACHE[key]


def _prep_in_maps(X, W_in1, b_in1, W_rec1, W_in2, b_in2, W_rec2, L_steps):
    with_bias = bool(np.any(b_in1) or np.any(b_in2))
    w1t = np.ascontiguousarray(W_rec1.T.astype(np.float32))
    w2t = np.ascontiguousarray(W_rec2.T.astype(np.float32))
    wi1t = np.ascontiguousarray(W_in1.T.astype(np.float32))
    wi2t = np.ascontiguousarray(W_in2.T.astype(np.float32))
    idn = np.eye(64, dtype=np.float32)
    zro = np.zeros((128, KH * BC), np.float32)
    in_maps = []
    for c in range(NC):
        xt = np.ascontiguousarray(
            X[c*BC:(c+1)*BC, :L_steps, :].transpose(1, 2, 0).astype(np.float32))
        m = {"XT": xt, "W1T": w1t, "W2T": w2t, "Wi1T": wi1t, "Wi2T": wi2t,
             "IDN": idn, "ZRO": zro}
        if with_bias:
            m["BIA"] = np.ascontiguousarray(
                np.stack([b_in1[:, 0], b_in2[:, 0]]).astype(np.float32))
            m["ONE"] = np.ones((1, BC), np.float32)
        in_maps.append(m)
    return in_maps, with_bias


def run_device(X, W_in1, b_in1, W_rec1, W_in2, b_in2, W_rec2, L_steps=L):
    """Run the recurrence on 8 cores; returns z1_final (B, H) float32."""
    from concourse.bass_utils import run_bass_kernel_spmd
    in_maps, with_bias = _prep_in_maps(X, W_in1, b_in1, W_rec1, W_in2, b_in2, W_rec2, L_steps)
    nc = _get_nc(L_steps, with_bias)
    res = run_bass_kernel_spmd(nc, in_maps, list(range(NC)))
    return np.concatenate([res.results[c]["OUT"] for c in range(NC)], axis=0)


def kernel(X, W_in1, b_in1, W_rec1, W_in2, b_in2, W_rec2, W_out, b_out):
    X = np.asarray(X); W_out = np.asarray(W_out); b_out = np.asarray(b_out)
    assert X.shape == (B, L, I), f"unexpected X shape {X.shape}"
    z1 = run_device(X, np.asarray(W_in1), np.asarray(b_in1),
                    np.asarray(W_rec1), np.asarray(W_in2), np.asarray(b_in2),
                    np.asarray(W_rec2))
    out = np.tanh(z1.astype(np.float64) @ W_out.astype(np.float64).T
                  + b_out.astype(np.float64)[:, 0])
    return out.reshape(B, 1).astype(np.float32)
